# revision 49
# baseline (speedup 1.0000x reference)
"""Trainium2 Bass kernel: causal multi-head attention with RoPE.

Reference computation (B=2, T=2048, C=2048, H=16, D=128, fp32):
    q/k/v = hs @ {q,k,v}_w^T ; RoPE(q), RoPE(k)
    out   = softmax(causal(q k^T / sqrt(D))) v @ o_w^T

Sharding: tensor-parallel over heads — each of the 8 cores owns 2 heads.
Each core computes its heads' projections + attention and a partial output
projection; the host sums the 8 partials.

Per-core device pipeline (all matmuls in float32r = full-rate fp32):
  A) stream hs^T chunks; qT/kT in [d, t] layout (per-window tiles so
     later stages only wait on the exact window they read), v in [t, d]
     layout; RoPE (rotate_half as a constant +-1 permutation matmul +
     cos/sin elementwise) interleaved per pair of chunks.
  B) scores computed TRANSPOSED [tk, tq]; exp on ACT with 1/sqrt(D)
     folded into the activation scale; causal masking as a 0/1 multiply
     on block-diagonal tiles only; softmax denominator via an M=1
     all-ones matmul accumulated alongside PV; unnormalized attnT
     overwrites the spent q window tiles.  Per head: reciprocal of the
     denominators via exp(-ln(x)) (one ACT table-set switch pair per
     head, overlapped with the next head's attention), gpsimd
     partition-broadcast, normalize in place.
  C) output projection interleaved with the last head's normalizes;
     partial [t, c] tiles to DRAM.
"""

import math
import sys

if "/opt/trn_rl_repo" not in sys.path:
    sys.path.insert(0, "/opt/trn_rl_repo")

import ml_dtypes
import numpy as np

import concourse.bass as bass
import concourse.mybir as mybir
import concourse.tile as tile
from concourse import bacc, bass_utils

F32 = mybir.dt.float32
F32R = mybir.dt.float32r
BF16 = mybir.dt.bfloat16
NP_BF16 = ml_dtypes.bfloat16
AF = mybir.ActivationFunctionType
MULT = mybir.AluOpType.mult
ADD = mybir.AluOpType.add

B = 2
C = 2048
H = 16
D = 128
N_CORES = 8
HPC = H // N_CORES  # heads per core
DPC = HPC * D  # channels per core (256)
ROPE_BASE = 10000.0
P = 128  # partitions
TQW = 512  # tq window (matmul free dim)
TCH = 256  # hs^T chunk width in t


def _build_nc(T: int = 2048):
    """Build the per-core Bass program (SPMD: same program, per-core data)."""
    KT = C // P  # 16 k-tiles over the contraction dim c
    n_ch = T // TCH  # hs chunks per batch
    n_w = T // TQW  # tq windows per (b, h)
    cpw = TQW // TCH  # chunks per window
    scale = 1.0 / math.sqrt(D)

    nc = bacc.Bacc(trn_type="TRN2", target_bir_lowering=False, debug=False)

    hst = nc.dram_tensor("hst", [B, P, T // TQW, KT // 4, 4, TQW], BF16, kind="ExternalInput").ap()
    wq = nc.dram_tensor("wq_t", [P, KT, DPC], BF16, kind="ExternalInput").ap()
    wk = nc.dram_tensor("wk_t", [P, KT, DPC], BF16, kind="ExternalInput").ap()
    wv = nc.dram_tensor("wv_t", [P, KT, DPC], BF16, kind="ExternalInput").ap()
    ow = nc.dram_tensor("ow_t", [P, HPC, C], BF16, kind="ExternalInput").ap()
    cos_d = nc.dram_tensor("cos_t", [D, T], BF16, kind="ExternalInput").ap()
    sin_d = nc.dram_tensor("sin_t", [D, T], BF16, kind="ExternalInput").ap()
    rp_d = nc.dram_tensor("rperm", [D, D], BF16, kind="ExternalInput").ap()
    ones_d = nc.dram_tensor("ones", [P, 1], BF16, kind="ExternalInput").ap()
    onesr_d = nc.dram_tensor("ones_r", [1, P], BF16, kind="ExternalInput").ap()
    msk_d = nc.dram_tensor("masks", [P, P], BF16, kind="ExternalInput").ap()
    out_d = nc.dram_tensor("out_p", [B, T // P, C // TQW, P, TQW], BF16, kind="ExternalOutput").ap()

    with tile.TileContext(nc) as tc:
        with (
            tc.tile_pool(name="consts", bufs=1) as consts,
            tc.tile_pool(name="hst", bufs=4) as hstp,
            tc.tile_pool(name="qkv", bufs=1) as qkvp,
            tc.tile_pool(name="exp", bufs=8) as expp,
            tc.tile_pool(name="small", bufs=2) as smallp,
            tc.tile_pool(name="acc", bufs=2) as accp,
            tc.tile_pool(name="bc", bufs=4) as bcp,
            tc.tile_pool(name="outp", bufs=3) as outp,
            tc.tile_pool(name="ps", bufs=8, space="PSUM") as ps,
        ):
            # ---- resident constants -------------------------------------
            wq_sb = [
                consts.tile([P, 4, DPC], BF16, tag=f"wq{i}", name=f"wq{i}")
                for i in range(KT // 4)
            ]
            wk_sb = [
                consts.tile([P, 4, DPC], BF16, tag=f"wk{i}", name=f"wk{i}")
                for i in range(KT // 4)
            ]
            wv_sb = [
                consts.tile([P, 4, DPC], BF16, tag=f"wv{i}", name=f"wv{i}")
                for i in range(KT // 4)
            ]
            ow_sb = consts.tile([P, HPC, C], BF16, tag="ow")
            cos_sb = consts.tile([D, T], BF16, tag="cos")
            sin_sb = consts.tile([D, T], BF16, tag="sin")
            msk_sb = consts.tile([P, P], BF16, tag="msk")
            ones_sb = consts.tile([P, 1], BF16, tag="ones")
            onesr_sb = consts.tile([1, P], BF16, tag="onesr")
            rp_sb = consts.tile([D, D], BF16, tag="rp")
            # Critical-path-first DMA order: the first chunk's matmuls
            # need only weight quarter 0 + hs chunk 0; everything else can
            # stream in behind them (queues drain FIFO).
            for w_sb, w_d in ((wq_sb, wq), (wk_sb, wk), (wv_sb, wv)):
                nc.sync.dma_start(w_sb[0][:], w_d[:, bass.ts(0, 4), :])
            pre_tiles = {}
            for qi in range(4):
                ht = hstp.tile([P, 4, TQW], BF16, tag="hst", name="ht_pre")
                nc.sync.dma_start(ht[:], hst[0, :, 0, qi, :, :])
                pre_tiles[qi] = ht
            for i in (1, 2, 3):
                for w_sb, w_d in ((wq_sb, wq), (wk_sb, wk), (wv_sb, wv)):
                    nc.sync.dma_start(w_sb[i][:], w_d[:, bass.ts(i, 4), :])
            nc.sync.dma_start(cos_sb[:], cos_d)
            nc.sync.dma_start(sin_sb[:], sin_d)
            nc.sync.dma_start(rp_sb[:], rp_d)
            late_dmas_done = []

            # Per-(b) q/k/v tiles; bufs=2 so batch 1's projections can be
            # emitted (and scheduled) while batch 0's attention still reads
            # the first buffer generation.
            q_t, k_t, v_sb = {}, {}, {}

            def alloc_qkv(b):
                q_t[b] = [
                    [
                        qkvp.tile([P, TQW], BF16, tag=f"q{h}w{w}", name=f"q{h}w{w}", bufs=2)
                        for w in range(n_w)
                    ]
                    for h in range(HPC)
                ]
                k_t[b] = [
                    [
                        qkvp.tile([P, TQW], BF16, tag=f"k{h}w{w}", name=f"k{h}w{w}", bufs=2)
                        for w in range(n_w)
                    ]
                    for h in range(HPC)
                ]
                v_sb[b] = qkvp.tile([P, T // P, DPC], BF16, tag="v", name="v", bufs=2)

            # rotate_half as an unsigned permutation matmul (PE) with the
            # sign folded into the sin table; ACT copies the psum to bf16
            # so the DVE ops run all-16-bit.
            def rope(b, w):
                sl = bass.ts(w, TQW)
                for h in range(HPC):
                    for x_t in (q_t[b], k_t[b]):
                        x = x_t[h][w]
                        rh = ps.tile([P, TQW], F32, tag="ps", name="rh")
                        nc.tensor.matmul(rh[:], rp_sb[:], x[:], start=True, stop=True)
                        rhs = smallp.tile([P, TQW], BF16, tag="rhs", name="rhs")
                        nc.scalar.activation(rhs[:], rh[:], AF.Copy)
                        t1 = smallp.tile([P, TQW], BF16, tag="t1")
                        nc.vector.tensor_tensor(t1[:], x[:], cos_sb[:, sl], op=MULT)
                        nc.vector.tensor_tensor(rhs[:], rhs[:], sin_sb[:, sl], op=MULT)
                        nc.vector.tensor_tensor(x[:], t1[:], rhs[:], op=ADD)

            # ---- phase A: one projection window ------------------------
            def proj_window(b, w):
                hts = []
                for qi in range(4):
                    if b == 0 and w == 0 and qi in pre_tiles:
                        ht = pre_tiles.pop(qi)
                    else:
                        ht = hstp.tile([P, 4, TQW], BF16, tag="hst", name="ht")
                        nc.sync.dma_start(ht[:], hst[b, :, w, qi, :, :])
                    hts.append(ht)
                pq = [ps.tile([P, TQW], F32, tag="ps", name="pq") for _ in range(HPC)]
                pk = [ps.tile([P, TQW], F32, tag="ps", name="pk") for _ in range(HPC)]
                for k in range(KT):
                    for h in range(HPC):
                        for pt, w_sb in ((pq[h], wq_sb), (pk[h], wk_sb)):
                            nc.tensor.matmul(
                                pt[:],
                                w_sb[k // 4][:, k % 4, bass.ts(h, D)],
                                hts[k // 4][:, k % 4, :],
                                start=(k == 0),
                                stop=(k == KT - 1),
                            )
                # Rank the psum->sbuf copies later so attention's exps win
                # the ACT queue (deps still force the copies on time).
                with tc.high_priority(-2000):
                    for h in range(HPC):
                        nc.scalar.activation(q_t[b][h][w][:], pq[h][:], AF.Copy)
                        nc.scalar.activation(k_t[b][h][w][:], pk[h][:], AF.Copy)
                pv4 = [
                    ps.tile([P, DPC], F32, tag="ps", name="pv4")
                    for _ in range(TQW // P)
                ]
                for k in range(KT):
                    for sub in range(TQW // P):
                        nc.tensor.matmul(
                            pv4[sub][:],
                            hts[k // 4][:, k % 4, bass.ts(sub, P)],
                            wv_sb[k // 4][:, k % 4, :],
                            start=(k == 0),
                            stop=(k == KT - 1),
                        )
                with tc.high_priority(-2000):
                    for sub in range(TQW // P):
                        nc.scalar.activation(
                            v_sb[b][:, w * (TQW // P) + sub, :], pv4[sub][:], AF.Copy
                        )
                rope(b, w)

            # ---- phase B: attention, two heads braided -----------------
            # Per braided iteration the PE does QK+PV for BOTH heads while
            # ACT runs the two exps; phase-C matmul groups of the previous
            # window are sprinkled in as PE filler.  The softmax
            # denominator accumulates on the DVE (acc += e, bf16 2x rate)
            # and collapses to a single ones-matmul per (h, w).
            FIFO = 2

            def attend_pair(b, w, filler, pops=2, fifo_n=FIFO):
                ntk = (w + 1) * (TQW // P)

                def qk_exp(h, i):
                    # Diagonal k-tiles (the last TQW//P of the window) only
                    # have valid queries q >= off*P: restrict QK / exp /
                    # mask / PV to those columns.
                    off = i - w * (TQW // P)
                    q0 = max(off, 0) * P
                    st = ps.tile([P, TQW], F32, tag="ps", name="st")
                    nc.tensor.matmul(
                        st[:, q0:],
                        k_t[b][h][i // (TQW // P)][:, bass.ts(i % (TQW // P), P)],
                        q_t[b][h][w][:, q0:],
                        start=True,
                        stop=True,
                    )
                    e = expp.tile([P, TQW], BF16, tag="exp")
                    nc.scalar.activation(e[:, q0:], st[:, q0:], AF.Exp, scale=scale)
                    if off >= 0:
                        nc.vector.tensor_tensor(
                            e[:, q0 : q0 + P], e[:, q0 : q0 + P], msk_sb[:], op=MULT
                        )
                    return e, q0

                acc = [
                    accp.tile([P, TQW], BF16, tag=f"acc{h}", name=f"acc{h}")
                    for h in range(HPC)
                ]
                pv = [
                    ps.tile([P, TQW], F32, tag="ps", name=f"pv{h}")
                    for h in range(HPC)
                ]
                fifo = [[], []]
                for j in range(min(fifo_n, ntk)):
                    for h in range(HPC):
                        fifo[h].append(qk_exp(h, j))
                for i in range(ntk):
                    for h in range(HPC):
                        if i + fifo_n < ntk:
                            fifo[h].append(qk_exp(h, i + fifo_n))
                        e, q0 = fifo[h].pop(0)
                        nc.tensor.matmul(
                            pv[h][:, q0:],
                            v_sb[b][:, i, bass.ts(h, D)],
                            e[:, q0:],
                            start=(i == 0),
                            stop=(i == ntk - 1),
                        )
                        if i == 0:
                            nc.vector.tensor_copy(acc[h][:], e[:])
                        else:
                            nc.vector.tensor_tensor(
                                acc[h][:, q0:], acc[h][:, q0:], e[:, q0:], op=ADD
                            )
                    for _ in range(pops):
                        if filler:
                            filler.pop(0)()
                # window end: denominator ones-matmul, early psum copy,
                # recip -> broadcast -> normalize in SBUF (bf16 2x).
                for h in range(HPC):
                    den = ps.tile([P, TQW], F32, tag="ps", name="den")
                    nc.tensor.matmul(
                        den[:1, :], ones_sb[:], acc[h][:], start=True, stop=True
                    )
                    nc.vector.tensor_copy(q_t[b][h][w][:], pv[h][:])
                    # reciprocal straight off the den psum row, then a K=1
                    # ones-row matmul replicates 1/den across partitions
                    # (PE, ~0.3us) instead of the gpsimd broadcast chain.
                    bcs = smallp.tile([1, TQW], F32, tag="bcs", name="bcs")
                    scr = smallp.tile([1, TQW], F32, tag="scr", name="scr")
                    nc.vector.reciprocal_approx_accurate(
                        out=bcs[:], in_=den[:1, :], scratch=scr[:]
                    )
                    bcb = smallp.tile([1, TQW], BF16, tag="bcb", name="bcb")
                    nc.vector.tensor_copy(bcb[:], bcs[:])
                    bcps = ps.tile([P, TQW], F32, tag="ps", name="bcps")
                    nc.tensor.matmul(
                        bcps[:], onesr_sb[:], bcb[:], start=True, stop=True
                    )
                    nc.vector.tensor_tensor(
                        q_t[b][h][w][:], q_t[b][h][w][:], bcps[:], op=MULT
                    )

            def phase_c_thunks(b, w):
                thunks = []
                for m in range(w * (TQW // P), (w + 1) * (TQW // P)):
                    for n in range(C // TQW):
                        def group(m=m, n=n, b=b):
                            po = ps.tile([P, TQW], F32, tag="ps", name="po")
                            for h in range(HPC):
                                nc.tensor.matmul(
                                    po[:],
                                    q_t[b][h][m // (TQW // P)][
                                        :, bass.ts(m % (TQW // P), P)
                                    ],
                                    ow_sb[:, h, bass.ts(n, TQW)],
                                    start=(h == 0),
                                    stop=(h == HPC - 1),
                                )
                            o_t = outp.tile([P, TQW], BF16, tag="o")
                            nc.any.tensor_copy(o_t[:], po[:])
                            nc.sync.dma_start(out_d[b, m, n], o_t[:])
                        thunks.append(group)
                return thunks

            # ---- emission: A0 | BC0 interleaved with A1 | BC1 ----------
            # w1 first in each BC: its leading tiles are unmasked and its
            # rope finished long ago, so the phase-A tail overlaps.
            wins = [1, 0] + list(range(2, n_w)) if n_w > 1 else [0]

            alloc_qkv(0)
            with nc.named_scope("A0"):
                for w in range(n_w):
                    proj_window(0, w)
                nc.sync.dma_start(msk_sb[:], msk_d)
                nc.sync.dma_start(ones_sb[:], ones_d)
                nc.sync.dma_start(onesr_sb[:], onesr_d)
                nc.sync.dma_start(ow_sb[:], ow)
            alloc_qkv(1)
            filler = []
            with nc.named_scope("BC0A1"):
                for idx, w in enumerate(wins):
                    attend_pair(0, w, filler)
                    for th in filler:  # drain leftover phase C
                        th()
                    filler = phase_c_thunks(0, w)
                    # batch 1's projection window: dense, dependency-free
                    # PE work the scheduler can use to fill attention
                    # stalls of batch 0.
                    proj_window(1, idx)
            with nc.named_scope("BC1"):
                for idx, w in enumerate(wins):
                    attend_pair(1, w, filler, fifo_n=3)
                    for th in filler:
                        th()
                    filler = phase_c_thunks(1, w)
                for th in filler:
                    th()

    nc.compile()
    return nc


def _host_prep(hidden_states, q_w, k_w, v_w, o_w):
    """Build the 8 per-core input maps (and shared constant tensors)."""
    T = hidden_states.shape[1]
    bf16 = NP_BF16

    n_w = T // TQW
    KT = C // P
    # [B, T, C] -> hs^T blocked per (partition, window, k-quarter):
    # [B, P, n_w, KT//4, 4, TQW]
    hstT = hidden_states.transpose(0, 2, 1)  # [B, C, T]
    hst = np.ascontiguousarray(
        hstT.reshape(B, KT // 4, 4, P, n_w, TQW).transpose(0, 3, 4, 1, 2, 5)
    ).astype(bf16)

    def wblk(w_slice):
        # [DPC, C] row-slice -> w^T blocked [P, KT, DPC]
        return np.ascontiguousarray(
            w_slice.T.reshape(KT, P, DPC).transpose(1, 0, 2)
        ).astype(bf16)


    inv_freq = 1.0 / (ROPE_BASE ** (np.arange(0, D, 2, dtype=np.float64) / D))
    t_ar = np.arange(T, dtype=np.float64)
    freqs = t_ar[:, None] * inv_freq[None, :]  # [T, D/2]
    cos_td = np.concatenate([np.cos(freqs), np.cos(freqs)], axis=-1)  # [T, D]
    sin_td = np.concatenate([np.sin(freqs), np.sin(freqs)], axis=-1)
    cos_t = np.ascontiguousarray(cos_td.T).astype(bf16)  # [D, T]
    # rotate_half is done on-device as a partition-permute DMA (pure
    # gather); the sign of the rotated half lives in the sin table:
    # rope = x*cos + gather(x)*sin_signed, sign = -1 for rows < D/2.
    sin_t = np.ascontiguousarray(sin_td.T).astype(np.float64)
    sin_t[: D // 2, :] *= -1.0
    sin_t = sin_t.astype(bf16)

    # unsigned rotate_half permutation (signs live in sin_t): rh = R @ x,
    # rperm = R^T as the lhsT operand.
    rperm = np.zeros((D, D), dtype=bf16)
    half = D // 2
    for j in range(half):
        rperm[2 * j + 1, j] = 1.0
    for j in range(half, D):
        rperm[2 * (j - half), j] = 1.0

    ones = np.ones((P, 1), dtype=bf16)
    ones_r = np.ones((1, P), dtype=bf16)

    # single [P, P] lower-triangular block mask for the diagonal k-tiles
    y = np.arange(P)[:, None]
    x = np.arange(P)[None, :]
    masks = (x >= y).astype(bf16)

    in_maps = []
    for c in range(N_CORES):
        rs, re = c * DPC, (c + 1) * DPC
        in_maps.append(
            {
                "hst": hst,
                "wq_t": wblk(q_w[rs:re, :]),
                "wk_t": wblk(k_w[rs:re, :]),
                "wv_t": wblk(v_w[rs:re, :]),
                "ow_t": np.ascontiguousarray(
                    o_w[:, rs:re].T.reshape(HPC, P, C).transpose(1, 0, 2)
                ).astype(bf16),
                "cos_t": cos_t,
                "sin_t": sin_t,
                "rperm": rperm,
                "ones": ones,
                "ones_r": ones_r,
                "masks": masks,
            }
        )
    return in_maps


_NC_CACHE = {}


def _get_nc(T):
    if T not in _NC_CACHE:
        _NC_CACHE[T] = _build_nc(T)
    return _NC_CACHE[T]


def kernel(hidden_states, q_w, k_w, v_w, o_w, **run_kwargs):
    hidden_states = np.asarray(hidden_states, dtype=np.float32)
    q_w = np.asarray(q_w, dtype=np.float32)
    k_w = np.asarray(k_w, dtype=np.float32)
    v_w = np.asarray(v_w, dtype=np.float32)
    o_w = np.asarray(o_w, dtype=np.float32)
    T = hidden_states.shape[1]
    nc = _get_nc(T)
    in_maps = _host_prep(hidden_states, q_w, k_w, v_w, o_w)
    res = bass_utils.run_bass_kernel_spmd(
        nc, in_maps, core_ids=list(range(N_CORES)), **run_kwargs
    )
    out = np.zeros((B, T // P, C // TQW, P, TQW), dtype=np.float64)
    for r in res.results:
        out += r["out_p"].astype(np.float64)
    kernel.last_results = res
    return (
        out.transpose(0, 1, 3, 2, 4).reshape(B, T, C).astype(np.float32)
    )



# revision 51
# speedup vs baseline: 1.1419x; 1.1419x over previous
"""Trainium2 Bass kernel: causal multi-head attention with RoPE.

Reference computation (B=2, T=2048, C=2048, H=16, D=128, fp32):
    q/k/v = hs @ {q,k,v}_w^T ; RoPE(q), RoPE(k)
    out   = softmax(causal(q k^T / sqrt(D))) v @ o_w^T

Sharding: tensor-parallel over heads — each of the 8 cores owns 2 heads.
Each core computes its heads' projections + attention and a partial output
projection; the host sums the 8 partials.

Per-core device pipeline (all matmuls in float32r = full-rate fp32):
  A) stream hs^T chunks; qT/kT in [d, t] layout (per-window tiles so
     later stages only wait on the exact window they read), v in [t, d]
     layout; RoPE (rotate_half as a constant +-1 permutation matmul +
     cos/sin elementwise) interleaved per pair of chunks.
  B) scores computed TRANSPOSED [tk, tq]; exp on ACT with 1/sqrt(D)
     folded into the activation scale; causal masking as a 0/1 multiply
     on block-diagonal tiles only; softmax denominator via an M=1
     all-ones matmul accumulated alongside PV; unnormalized attnT
     overwrites the spent q window tiles.  Per head: reciprocal of the
     denominators via exp(-ln(x)) (one ACT table-set switch pair per
     head, overlapped with the next head's attention), gpsimd
     partition-broadcast, normalize in place.
  C) output projection interleaved with the last head's normalizes;
     partial [t, c] tiles to DRAM.
"""

import math
import sys

if "/opt/trn_rl_repo" not in sys.path:
    sys.path.insert(0, "/opt/trn_rl_repo")

import ml_dtypes
import numpy as np

import concourse.bass as bass
import concourse.mybir as mybir
import concourse.tile as tile
from concourse import bacc, bass_utils

F32 = mybir.dt.float32
F32R = mybir.dt.float32r
BF16 = mybir.dt.bfloat16
NP_BF16 = ml_dtypes.bfloat16
AF = mybir.ActivationFunctionType
MULT = mybir.AluOpType.mult
ADD = mybir.AluOpType.add

B = 2
C = 2048
H = 16
D = 128
N_CORES = 8
HPC = H // N_CORES  # heads per core
DPC = HPC * D  # channels per core (256)
ROPE_BASE = 10000.0
P = 128  # partitions
TQW = 512  # tq window (matmul free dim)
TCH = 256  # hs^T chunk width in t


def _build_nc(T: int = 2048):
    """Build the per-core Bass program (SPMD: same program, per-core data)."""
    KT = C // P  # 16 k-tiles over the contraction dim c
    n_ch = T // TCH  # hs chunks per batch
    n_w = T // TQW  # tq windows per (b, h)
    cpw = TQW // TCH  # chunks per window
    scale = 1.0 / math.sqrt(D)

    nc = bacc.Bacc(trn_type="TRN2", target_bir_lowering=False, debug=False)

    hst = nc.dram_tensor("hst", [B, P, T // TQW, KT // 4, 4, TQW], BF16, kind="ExternalInput").ap()
    wq = nc.dram_tensor("wq_t", [P, KT, DPC], BF16, kind="ExternalInput").ap()
    wk = nc.dram_tensor("wk_t", [P, KT, DPC], BF16, kind="ExternalInput").ap()
    wv = nc.dram_tensor("wv_t", [P, KT, DPC], BF16, kind="ExternalInput").ap()
    ow = nc.dram_tensor("ow_t", [P, HPC, C], BF16, kind="ExternalInput").ap()
    cos_d = nc.dram_tensor("cos_t", [D, T], BF16, kind="ExternalInput").ap()
    sin_d = nc.dram_tensor("sin_t", [D, T], BF16, kind="ExternalInput").ap()
    rp_d = nc.dram_tensor("rperm", [D, D], BF16, kind="ExternalInput").ap()
    ones_d = nc.dram_tensor("ones", [P, 1], BF16, kind="ExternalInput").ap()
    msk_d = nc.dram_tensor("masks", [P, P], BF16, kind="ExternalInput").ap()
    out_d = nc.dram_tensor("out_p", [B, T // P, C // TQW, P, TQW], BF16, kind="ExternalOutput").ap()

    with tile.TileContext(nc) as tc:
        with (
            tc.tile_pool(name="consts", bufs=1) as consts,
            tc.tile_pool(name="hst", bufs=4) as hstp,
            tc.tile_pool(name="qkv", bufs=1) as qkvp,
            tc.tile_pool(name="exp", bufs=8) as expp,
            tc.tile_pool(name="small", bufs=2) as smallp,
            tc.tile_pool(name="acc", bufs=2) as accp,
            tc.tile_pool(name="bc", bufs=4) as bcp,
            tc.tile_pool(name="outp", bufs=6) as outp,
            tc.tile_pool(name="ps", bufs=8, space="PSUM") as ps,
        ):
            # ---- resident constants -------------------------------------
            wq_sb = [
                consts.tile([P, 4, DPC], BF16, tag=f"wq{i}", name=f"wq{i}")
                for i in range(KT // 4)
            ]
            wk_sb = [
                consts.tile([P, 4, DPC], BF16, tag=f"wk{i}", name=f"wk{i}")
                for i in range(KT // 4)
            ]
            wv_sb = [
                consts.tile([P, 4, DPC], BF16, tag=f"wv{i}", name=f"wv{i}")
                for i in range(KT // 4)
            ]
            ow_sb = consts.tile([P, HPC, C], BF16, tag="ow")
            cos_sb = consts.tile([D, T], BF16, tag="cos")
            sin_sb = consts.tile([D, T], BF16, tag="sin")
            msk_sb = consts.tile([P, P], BF16, tag="msk")
            ones_sb = consts.tile([P, 1], BF16, tag="ones")
            rp_sb = consts.tile([D, D], BF16, tag="rp")
            # Critical-path-first DMA order: the first chunk's matmuls
            # need only weight quarter 0 + hs chunk 0; everything else can
            # stream in behind them (queues drain FIFO).
            for w_sb, w_d in ((wq_sb, wq), (wk_sb, wk), (wv_sb, wv)):
                nc.sync.dma_start(w_sb[0][:], w_d[:, bass.ts(0, 4), :])
            pre_tiles = {}
            for qi in range(4):
                ht = hstp.tile([P, 4, TQW], BF16, tag="hst", name="ht_pre")
                nc.sync.dma_start(ht[:], hst[0, :, 0, qi, :, :])
                pre_tiles[qi] = ht
            for i in (1, 2, 3):
                for w_sb, w_d in ((wq_sb, wq), (wk_sb, wk), (wv_sb, wv)):
                    nc.sync.dma_start(w_sb[i][:], w_d[:, bass.ts(i, 4), :])
            nc.sync.dma_start(cos_sb[:], cos_d)
            nc.sync.dma_start(sin_sb[:], sin_d)
            nc.sync.dma_start(rp_sb[:], rp_d)
            late_dmas_done = []

            # Per-(b) q/k/v tiles; bufs=2 so batch 1's projections can be
            # emitted (and scheduled) while batch 0's attention still reads
            # the first buffer generation.
            q_t, k_t, v_sb = {}, {}, {}

            def alloc_qkv(b):
                q_t[b] = [
                    [
                        qkvp.tile([P, TQW], BF16, tag=f"q{h}w{w}", name=f"q{h}w{w}", bufs=2)
                        for w in range(n_w)
                    ]
                    for h in range(HPC)
                ]
                k_t[b] = [
                    [
                        qkvp.tile([P, TQW], BF16, tag=f"k{h}w{w}", name=f"k{h}w{w}", bufs=2)
                        for w in range(n_w)
                    ]
                    for h in range(HPC)
                ]
                v_sb[b] = qkvp.tile([P, T // P, DPC], BF16, tag="v", name="v", bufs=2)

            # rotate_half as an unsigned permutation matmul (PE) with the
            # sign folded into the sin table; ACT copies the psum to bf16
            # so the DVE ops run all-16-bit.
            def rope(b, w):
                sl = bass.ts(w, TQW)
                for h in range(HPC):
                    for x_t in (q_t[b], k_t[b]):
                        x = x_t[h][w]
                        rh = ps.tile([P, TQW], F32, tag="ps", name="rh")
                        nc.tensor.matmul(rh[:], rp_sb[:], x[:], start=True, stop=True)
                        rhs = smallp.tile([P, TQW], BF16, tag="rhs", name="rhs")
                        nc.scalar.activation(rhs[:], rh[:], AF.Copy)
                        t1 = smallp.tile([P, TQW], BF16, tag="t1")
                        nc.vector.tensor_tensor(t1[:], x[:], cos_sb[:, sl], op=MULT)
                        nc.vector.tensor_tensor(rhs[:], rhs[:], sin_sb[:, sl], op=MULT)
                        nc.vector.tensor_tensor(x[:], t1[:], rhs[:], op=ADD)

            # ---- phase A: one projection window ------------------------
            def proj_window(b, w):
                hts = []
                for qi in range(4):
                    if b == 0 and w == 0 and qi in pre_tiles:
                        ht = pre_tiles.pop(qi)
                    else:
                        ht = hstp.tile([P, 4, TQW], BF16, tag="hst", name="ht")
                        nc.sync.dma_start(ht[:], hst[b, :, w, qi, :, :])
                    hts.append(ht)
                pq = [ps.tile([P, TQW], F32, tag="ps", name="pq") for _ in range(HPC)]
                pk = [ps.tile([P, TQW], F32, tag="ps", name="pk") for _ in range(HPC)]
                for k in range(KT):
                    for h in range(HPC):
                        for pt, w_sb in ((pq[h], wq_sb), (pk[h], wk_sb)):
                            nc.tensor.matmul(
                                pt[:],
                                w_sb[k // 4][:, k % 4, bass.ts(h, D)],
                                hts[k // 4][:, k % 4, :],
                                start=(k == 0),
                                stop=(k == KT - 1),
                            )
                # Rank the psum->sbuf copies later so attention's exps win
                # the ACT queue (deps still force the copies on time).
                with tc.high_priority(-2000):
                    for h in range(HPC):
                        nc.scalar.activation(q_t[b][h][w][:], pq[h][:], AF.Copy)
                        nc.scalar.activation(k_t[b][h][w][:], pk[h][:], AF.Copy)
                pv4 = [
                    ps.tile([P, DPC], F32, tag="ps", name="pv4")
                    for _ in range(TQW // P)
                ]
                for k in range(KT):
                    for sub in range(TQW // P):
                        nc.tensor.matmul(
                            pv4[sub][:],
                            hts[k // 4][:, k % 4, bass.ts(sub, P)],
                            wv_sb[k // 4][:, k % 4, :],
                            start=(k == 0),
                            stop=(k == KT - 1),
                        )
                with tc.high_priority(-2000):
                    for sub in range(TQW // P):
                        nc.scalar.activation(
                            v_sb[b][:, w * (TQW // P) + sub, :], pv4[sub][:], AF.Copy
                        )
                rope(b, w)

            # ---- phase B: attention, two heads braided -----------------
            # Per braided iteration the PE does QK+PV for BOTH heads while
            # ACT runs the two exps; phase-C matmul groups of the previous
            # window are sprinkled in as PE filler.  The softmax
            # denominator accumulates on the DVE (acc += e, bf16 2x rate)
            # and collapses to a single ones-matmul per (h, w).
            FIFO = 2

            def attend_pair(b, w, filler, pops=2, fifo_n=FIFO):
                ntk = (w + 1) * (TQW // P)

                def qk_exp(h, i):
                    # Diagonal k-tiles (the last TQW//P of the window) only
                    # have valid queries q >= off*P: restrict QK / exp /
                    # mask / PV to those columns.
                    off = i - w * (TQW // P)
                    q0 = max(off, 0) * P
                    st = ps.tile([P, TQW], F32, tag="ps", name="st")
                    nc.tensor.matmul(
                        st[:, q0:],
                        k_t[b][h][i // (TQW // P)][:, bass.ts(i % (TQW // P), P)],
                        q_t[b][h][w][:, q0:],
                        start=True,
                        stop=True,
                    )
                    e = expp.tile([P, TQW], BF16, tag="exp")
                    nc.scalar.activation(e[:, q0:], st[:, q0:], AF.Exp, scale=scale)
                    if off >= 0:
                        nc.vector.tensor_tensor(
                            e[:, q0 : q0 + P], e[:, q0 : q0 + P], msk_sb[:], op=MULT
                        )
                    return e, q0

                acc = [
                    accp.tile([P, TQW], BF16, tag=f"acc{h}", name=f"acc{h}")
                    for h in range(HPC)
                ]
                pv = [
                    ps.tile([P, TQW], F32, tag="ps", name=f"pv{h}")
                    for h in range(HPC)
                ]
                fifo = [[], []]
                for j in range(min(fifo_n, ntk)):
                    for h in range(HPC):
                        fifo[h].append(qk_exp(h, j))
                for i in range(ntk):
                    for h in range(HPC):
                        if i + fifo_n < ntk:
                            fifo[h].append(qk_exp(h, i + fifo_n))
                        e, q0 = fifo[h].pop(0)
                        nc.tensor.matmul(
                            pv[h][:, q0:],
                            v_sb[b][:, i, bass.ts(h, D)],
                            e[:, q0:],
                            start=(i == 0),
                            stop=(i == ntk - 1),
                        )
                        if i == 0:
                            nc.vector.tensor_copy(acc[h][:], e[:])
                        else:
                            nc.vector.tensor_tensor(
                                acc[h][:, q0:], acc[h][:, q0:], e[:, q0:], op=ADD
                            )
                    for _ in range(pops):
                        if filler:
                            filler.pop(0)()
                # window end: denominator ones-matmul, early psum copy,
                # recip -> broadcast -> normalize in SBUF (bf16 2x).
                for h in range(HPC):
                    den = ps.tile([P, TQW], F32, tag="ps", name="den")
                    nc.tensor.matmul(
                        den[:1, :], ones_sb[:], acc[h][:], start=True, stop=True
                    )
                    nc.vector.tensor_copy(q_t[b][h][w][:], pv[h][:])
                    bcs = smallp.tile([1, TQW], F32, tag="bcs", name="bcs")
                    scr = smallp.tile([1, TQW], F32, tag="scr", name="scr")
                    nc.vector.tensor_copy(bcs[:], den[:1, :])
                    nc.vector.reciprocal_approx_accurate(
                        out=bcs[:], in_=bcs[:], scratch=scr[:]
                    )
                    bcb = smallp.tile([1, TQW], BF16, tag="bcb", name="bcb")
                    nc.vector.tensor_copy(bcb[:], bcs[:])
                    bc = bcp.tile([P, TQW], BF16, tag="bc", name="bc")
                    nc.gpsimd.partition_broadcast(bc[:], bcb[:])
                    nc.vector.tensor_tensor(
                        q_t[b][h][w][:], q_t[b][h][w][:], bc[:], op=MULT
                    )

            def phase_c_thunks(b, w):
                thunks = []
                for m in range(w * (TQW // P), (w + 1) * (TQW // P)):
                    for n in range(C // TQW):
                        def group(m=m, n=n, b=b):
                            po = ps.tile([P, TQW], F32, tag="ps", name="po")
                            for h in range(HPC):
                                nc.tensor.matmul(
                                    po[:],
                                    q_t[b][h][m // (TQW // P)][
                                        :, bass.ts(m % (TQW // P), P)
                                    ],
                                    ow_sb[:, h, bass.ts(n, TQW)],
                                    start=(h == 0),
                                    stop=(h == HPC - 1),
                                )
                            o_t = outp.tile([P, TQW], BF16, tag="o")
                            nc.any.tensor_copy(o_t[:], po[:])
                            nc.sync.dma_start(out_d[b, m, n], o_t[:])
                        thunks.append(group)
                return thunks

            # ---- emission: A0 | BC0 interleaved with A1 | BC1 ----------
            # w1 first in each BC: its leading tiles are unmasked and its
            # rope finished long ago, so the phase-A tail overlaps.
            wins = [1, 0] + list(range(2, n_w)) if n_w > 1 else [0]

            alloc_qkv(0)
            with nc.named_scope("A0"):
                proj_window(0, 0)
                # mask/ones early: the w1 braid pulled into A0 needs them
                nc.sync.dma_start(msk_sb[:], msk_d)
                nc.sync.dma_start(ones_sb[:], ones_d)
                proj_window(0, 1)
                proj_window(0, 2)
                # first attention braid early: its exp/DVE warmup hides
                # under the dense PE stream of A0's last window.
                attend_pair(0, 1, [])
                proj_window(0, 3)
                nc.sync.dma_start(ow_sb[:], ow)
            alloc_qkv(1)
            filler = phase_c_thunks(0, 1)
            with nc.named_scope("BC0A1"):
                for idx, w in enumerate([0, 2, 3]):
                    attend_pair(0, w, filler)
                    for th in filler:  # drain leftover phase C
                        th()
                    filler = phase_c_thunks(0, w)
                    # batch 1's projection window: dense, dependency-free
                    # PE work the scheduler can use to fill attention
                    # stalls of batch 0.
                    proj_window(1, idx)
                proj_window(1, 3)
            with nc.named_scope("BC1"):
                for idx, w in enumerate(wins):
                    attend_pair(1, w, filler, fifo_n=3)
                    for th in filler:
                        th()
                    filler = phase_c_thunks(1, w)
                for th in filler:
                    th()

    nc.compile()
    return nc


def _host_prep(hidden_states, q_w, k_w, v_w, o_w):
    """Build the 8 per-core input maps (and shared constant tensors)."""
    T = hidden_states.shape[1]
    bf16 = NP_BF16

    n_w = T // TQW
    KT = C // P
    # [B, T, C] -> hs^T blocked per (partition, window, k-quarter):
    # [B, P, n_w, KT//4, 4, TQW]
    hstT = hidden_states.transpose(0, 2, 1)  # [B, C, T]
    hst = np.ascontiguousarray(
        hstT.reshape(B, KT // 4, 4, P, n_w, TQW).transpose(0, 3, 4, 1, 2, 5)
    ).astype(bf16)

    def wblk(w_slice):
        # [DPC, C] row-slice -> w^T blocked [P, KT, DPC]
        return np.ascontiguousarray(
            w_slice.T.reshape(KT, P, DPC).transpose(1, 0, 2)
        ).astype(bf16)


    inv_freq = 1.0 / (ROPE_BASE ** (np.arange(0, D, 2, dtype=np.float64) / D))
    t_ar = np.arange(T, dtype=np.float64)
    freqs = t_ar[:, None] * inv_freq[None, :]  # [T, D/2]
    cos_td = np.concatenate([np.cos(freqs), np.cos(freqs)], axis=-1)  # [T, D]
    sin_td = np.concatenate([np.sin(freqs), np.sin(freqs)], axis=-1)
    cos_t = np.ascontiguousarray(cos_td.T).astype(bf16)  # [D, T]
    # rotate_half is done on-device as a partition-permute DMA (pure
    # gather); the sign of the rotated half lives in the sin table:
    # rope = x*cos + gather(x)*sin_signed, sign = -1 for rows < D/2.
    sin_t = np.ascontiguousarray(sin_td.T).astype(np.float64)
    sin_t[: D // 2, :] *= -1.0
    sin_t = sin_t.astype(bf16)

    # unsigned rotate_half permutation (signs live in sin_t): rh = R @ x,
    # rperm = R^T as the lhsT operand.
    rperm = np.zeros((D, D), dtype=bf16)
    half = D // 2
    for j in range(half):
        rperm[2 * j + 1, j] = 1.0
    for j in range(half, D):
        rperm[2 * (j - half), j] = 1.0

    ones = np.ones((P, 1), dtype=bf16)

    # single [P, P] lower-triangular block mask for the diagonal k-tiles
    y = np.arange(P)[:, None]
    x = np.arange(P)[None, :]
    masks = (x >= y).astype(bf16)

    in_maps = []
    for c in range(N_CORES):
        rs, re = c * DPC, (c + 1) * DPC
        in_maps.append(
            {
                "hst": hst,
                "wq_t": wblk(q_w[rs:re, :]),
                "wk_t": wblk(k_w[rs:re, :]),
                "wv_t": wblk(v_w[rs:re, :]),
                "ow_t": np.ascontiguousarray(
                    o_w[:, rs:re].T.reshape(HPC, P, C).transpose(1, 0, 2)
                ).astype(bf16),
                "cos_t": cos_t,
                "sin_t": sin_t,
                "rperm": rperm,
                "ones": ones,
                "masks": masks,
            }
        )
    return in_maps


_NC_CACHE = {}


def _get_nc(T):
    if T not in _NC_CACHE:
        _NC_CACHE[T] = _build_nc(T)
    return _NC_CACHE[T]


def kernel(hidden_states, q_w, k_w, v_w, o_w, **run_kwargs):
    hidden_states = np.asarray(hidden_states, dtype=np.float32)
    q_w = np.asarray(q_w, dtype=np.float32)
    k_w = np.asarray(k_w, dtype=np.float32)
    v_w = np.asarray(v_w, dtype=np.float32)
    o_w = np.asarray(o_w, dtype=np.float32)
    T = hidden_states.shape[1]
    nc = _get_nc(T)
    in_maps = _host_prep(hidden_states, q_w, k_w, v_w, o_w)
    res = bass_utils.run_bass_kernel_spmd(
        nc, in_maps, core_ids=list(range(N_CORES)), **run_kwargs
    )
    out = np.zeros((B, T // P, C // TQW, P, TQW), dtype=np.float64)
    for r in res.results:
        out += r["out_p"].astype(np.float64)
    kernel.last_results = res
    return (
        out.transpose(0, 1, 3, 2, 4).reshape(B, T, C).astype(np.float32)
    )

